# revision 11
# baseline (speedup 1.0000x reference)
"""Edge-parallel GNN message-passing kernel for 8 trn2 NeuronCores.

Computation (see reference):
    p = x @ Wp + bp   [N,1]
    c = x @ Wc + bc   [N,1]
    out[e] = |p[dst[e]] - c[src[e]]| * W1 + b1   for each edge e

Strategy:
  - Node projection is sharded: core k projects nodes [12500k, 12500(k+1)).
    Bias is folded into the final affine ((p+bp)-(c+bc) = p-c+(bp-bc)).
  - An 8-core AllGather shares the per-node projections (800 KB total) so
    every core holds the full (p, c) table g[200704] in DRAM.
  - The table is re-laid out as T8[q, 0:8] = g[8q:8q+8] (rows 256 B apart)
    so the bulk SWDGE dma_gather instruction (256 B elements, int16 row
    indices < 32768) can fetch, for each edge endpoint, the 8-value row
    containing its projection; an 8-wide masked select on DVE picks the
    right value.
  - Edges are sharded contiguously, 75000 per core; per-edge row/offset
    indices are precomputed on the host. The fused sub/abs/affine tail
    runs on DVE/ACT and each core writes its output slice.
"""

import numpy as np

import concourse.bacc as bacc
import concourse.tile as tile
from concourse import bass, mybir
from concourse import bass_utils
from concourse.masks import make_identity

N_CORES = 8
N_NODES = 100000
N_EDGES = 600000
IN_CH = 128

NPC = 12500          # real nodes per core
NPC_PAD = 12544      # padded to 98*128
T_TILES = 98         # node tiles per core
EPC = 75000          # edges per core
S = 587              # free-dim width of edge tiles (128*587 = 75136)
EPC_PAD = 128 * S
G_FLAT = N_CORES * 2 * NPC_PAD   # 200704 table elements
T8_ROWS = G_FLAT // 8            # 25088 rows of 8 values (256B apart)
NI_CHUNK = 8192                  # dma_gather indices per instruction
IDX_COLS = EPC_PAD // 16         # 4696

F32 = mybir.dt.float32
I16 = mybir.dt.int16

_CACHED_NC = None
_LAST_RES = None


def _build_nc():
    nc = bacc.Bacc("TRN2", target_bir_lowering=False, debug=False,
                   num_devices=N_CORES)

    xs = nc.dram_tensor("xs", [NPC_PAD, IN_CH], F32, kind="ExternalInput")
    qd = nc.dram_tensor("qd", [128, IDX_COLS], I16, kind="ExternalInput")
    qs = nc.dram_tensor("qs", [128, IDX_COLS], I16, kind="ExternalInput")
    rd = nc.dram_tensor("rd", [128, S], F32, kind="ExternalInput")
    rs = nc.dram_tensor("rs", [128, S], F32, kind="ExternalInput")
    w = nc.dram_tensor("w", [IN_CH, 2], F32, kind="ExternalInput")
    scal = nc.dram_tensor("scal", [128, 16], F32, kind="ExternalInput")
    out = nc.dram_tensor("out", [EPC_PAD], F32, kind="ExternalOutput")
    import os as _os
    _dbg = _os.environ.get("K_DEBUG") == "1"
    if _dbg:
        out_vd = nc.dram_tensor("out_vd", [128, S], F32, kind="ExternalOutput")
        out_pd = nc.dram_tensor("out_pd", [128, S], F32, kind="ExternalOutput")
        out_vs = nc.dram_tensor("out_vs", [128, S], F32, kind="ExternalOutput")
        out_gt = nc.dram_tensor("out_gt", [N_CORES, 2, NPC_PAD], F32,
                                kind="ExternalOutput")
        out_sc = nc.dram_tensor("out_sc", [128, 16], F32, kind="ExternalOutput")

    with tile.TileContext(nc) as tc:
        with (
            tc.tile_pool(name="cst", bufs=1) as cst,
            tc.tile_pool(name="sb", bufs=3) as sb,
            tc.tile_pool(name="edge", bufs=1) as edge,
            tc.tile_pool(name="gat", bufs=3) as gat,
            tc.tile_pool(name="ps", bufs=2, space="PSUM") as ps,
            tc.tile_pool(name="pcps", bufs=1, space="PSUM") as pcps,
            tc.tile_pool(name="dram", bufs=1, space="DRAM") as dram,
        ):
            ident = cst.tile([128, 128], F32)
            make_identity(nc, ident[:])
            w_sb = cst.tile([IN_CH, 2], F32)
            nc.sync.dma_start(out=w_sb[:], in_=w[:])
            scal_sb = cst.tile([128, 16], F32)
            nc.sync.dma_start(out=scal_sb[:], in_=scal[:])

            qd_sb = edge.tile([128, IDX_COLS], I16)
            nc.sync.dma_start(out=qd_sb[:], in_=qd[:])
            qs_sb = edge.tile([128, IDX_COLS], I16)
            nc.sync.dma_start(out=qs_sb[:], in_=qs[:])
            rd_sb = edge.tile([128, S], F32)
            nc.sync.dma_start(out=rd_sb[:], in_=rd[:])
            rs_sb = edge.tile([128, S], F32)
            nc.sync.dma_start(out=rs_sb[:], in_=rs[:])

            # ---- phase 1: project this core's nodes: pc[n, 0:2] = x[n] @ [Wp|Wc]
            J = 7
            G = T_TILES // J
            xs_r = xs.rearrange("(g j p) c -> g p j c", j=J, p=128)
            pc_ps = pcps.tile([128, 2 * T_TILES], F32)
            for g in range(G):
                xt = sb.tile([128, J, IN_CH], F32, tag="xt")
                nc.sync.dma_start(out=xt[:], in_=xs_r[g])
                for j in range(J):
                    t = g * J + j
                    tp = ps.tile([128, 128], F32, tag="tp")
                    nc.tensor.transpose(tp[:], xt[:, j, :], ident[:])
                    x_t = sb.tile([128, 128], F32, tag="x_t")
                    nc.vector.tensor_copy(x_t[:], tp[:])
                    nc.tensor.matmul(
                        out=pc_ps[:, 2 * t:2 * t + 2],
                        lhsT=x_t[:],
                        rhs=w_sb[:],
                        start=True,
                        stop=True,
                    )
            pc_sb = cst.tile([128, 2 * T_TILES], F32)
            nc.vector.tensor_copy(pc_sb[:], pc_ps[:])

            # ---- phase 2: transpose p and c into node-contiguous rows
            bounce = dram.tile([2, NPC_PAD], F32)
            for comp in range(2):
                cp_ps = ps.tile([T_TILES, 128], F32, tag="cp")
                nc.tensor.transpose(
                    cp_ps[:], pc_sb[:, comp::2], ident[:]
                )
                row = sb.tile([T_TILES, 128], F32, tag="row")
                nc.vector.tensor_copy(row[:], cp_ps[:])
                nc.sync.dma_start(
                    out=bounce[comp].rearrange("(t p) -> t p", p=128),
                    in_=row[:],
                )

            # ---- phase 3: all-gather the projection table
            g_tab = dram.tile([N_CORES, 2, NPC_PAD], F32)
            nc.gpsimd.collective_compute(
                "AllGather",
                mybir.AluOpType.bypass,
                replica_groups=[list(range(N_CORES))],
                ins=[bounce.opt()],
                outs=[g_tab.opt()],
            )

            # ---- phase 3b: spread g into 256B-strided rows T8[q,0:8]=g[8q:8q+8]
            t8 = dram.tile([T8_ROWS, 64], F32)
            nc.sync.dma_start(
                out=t8[:, 0:8],
                in_=g_tab.rearrange("a b (q e) -> (a b q) e", e=8),
            )

            # ---- phase 4+5: bulk-gather endpoint rows, 8-wide select, tail
            val_d = edge.tile([128, S], F32)
            val_s = edge.tile([128, S], F32)
            res = edge.tile([128, S], F32)

            iota_b = scal_sb[:, 0:8]  # cols 0..7 hold 0..7
            n_full = EPC_PAD // NI_CHUNK          # 9 full chunks
            widths = [NI_CHUNK // 128] * n_full   # 64 columns each
            rem = EPC_PAD - n_full * NI_CHUNK
            if rem:
                widths.append(rem // 128)
            i0 = 0
            for wdt in widths:
                ni = wdt * 128
                icol0 = i0 * 8
                for qx_sb, rx_sb, vx in (
                    (qs_sb, rs_sb, val_s),
                    (qd_sb, rd_sb, val_d),
                ):
                    gth = gat.tile([128, NI_CHUNK // 128, 64], F32, tag="gth")
                    nc.gpsimd.dma_gather(
                        out_ap=gth[:, :wdt, :],
                        in_ap=t8[:],
                        idxs_ap=qx_sb[:, icol0:icol0 + wdt * 8],
                        num_idxs=ni,
                        num_idxs_reg=ni,
                        elem_size=64,
                        single_packet=False,
                    )
                    msk = gat.tile([128, NI_CHUNK // 128, 8], F32, tag="msk")
                    nc.vector.tensor_tensor(
                        out=msk[:, :wdt, :],
                        in0=iota_b.rearrange("p (one e) -> p one e", one=1).broadcast_to([128, wdt, 8]),
                        in1=rx_sb[:, i0:i0 + wdt].rearrange("p (i one) -> p i one", one=1).broadcast_to([128, wdt, 8]),
                        op=mybir.AluOpType.is_equal,
                    )
                    nc.vector.tensor_tensor(
                        out=msk[:, :wdt, :],
                        in0=msk[:, :wdt, :],
                        in1=gth[:, :wdt, 0:8],
                        op=mybir.AluOpType.mult,
                    )
                    nc.vector.tensor_reduce(
                        out=vx[:, i0:i0 + wdt],
                        in_=msk[:, :wdt, :],
                        axis=mybir.AxisListType.X,
                        op=mybir.AluOpType.add,
                    )
                if _dbg:
                    nc.sync.dma_start(out=out_pd[:, i0:i0 + wdt],
                                      in_=val_d[:, i0:i0 + wdt])
                # tail: |pd - cs + (bp-bc)| * w1 + b1
                sl = slice(i0, i0 + wdt)
                nc.vector.tensor_tensor(
                    out=val_d[:, sl], in0=val_d[:, sl], in1=val_s[:, sl],
                    op=mybir.AluOpType.subtract,
                )
                nc.scalar.activation(
                    out=val_d[:, sl], in_=val_d[:, sl],
                    func=mybir.ActivationFunctionType.Abs,
                    bias=scal_sb[:, 8:9], scale=1.0,
                )
                nc.vector.scalar_tensor_tensor(
                    out=res[:, sl], in0=val_d[:, sl],
                    scalar=scal_sb[:, 9:10],
                    in1=scal_sb[:, 10:11].to_broadcast([128, wdt]),
                    op0=mybir.AluOpType.mult,
                    op1=mybir.AluOpType.add,
                )
                i0 += wdt
            nc.sync.dma_start(
                out=out.rearrange("(p s) -> p s", s=S), in_=res[:]
            )
            if _dbg:
                nc.sync.dma_start(out=out_sc[:], in_=scal_sb[:])
                nc.sync.dma_start(out=out_vd[:], in_=val_d[:])
                nc.sync.dma_start(out=out_vs[:], in_=val_s[:])
                nc.sync.dma_start(out=out_gt[:], in_=g_tab[:])

    nc.compile()
    return nc


def _wrap16(stream):
    """idx j -> [j % 16, j // 16], replicated to all 8 gpsimd core groups."""
    w = stream.reshape(-1, 16).T  # [16, COLS]
    return np.tile(w, (8, 1))


def kernel(x, adjs, Wp, bp, Wc, bc, W1, b1):
    global _CACHED_NC
    x = np.ascontiguousarray(np.asarray(x, dtype=np.float32))
    adjs = np.asarray(adjs)
    Wp = np.asarray(Wp, dtype=np.float32)
    bp = np.asarray(bp, dtype=np.float32)
    Wc = np.asarray(Wc, dtype=np.float32)
    bc = np.asarray(bc, dtype=np.float32)
    W1 = np.asarray(W1, dtype=np.float32)
    b1 = np.asarray(b1, dtype=np.float32)

    src = adjs[0].astype(np.int64)
    dst = adjs[1].astype(np.int64)
    # flat indices into the gathered table g[core, comp, node_in_core]
    pidx = (dst // NPC) * (2 * NPC_PAD) + (dst % NPC)
    cidx = (src // NPC) * (2 * NPC_PAD) + NPC_PAD + (src % NPC)

    w = np.concatenate([Wp, Wc], axis=1)  # [128, 2]
    scal = np.zeros((128, 16), dtype=np.float32)
    scal[:, 0:8] = np.arange(8, dtype=np.float32)[None, :]
    scal[:, 8] = bp[0] - bc[0]
    scal[:, 9] = W1[0, 0]
    scal[:, 10] = b1[0]

    in_maps = []
    for k in range(N_CORES):
        xsl = np.zeros((NPC_PAD, IN_CH), dtype=np.float32)
        xsl[:NPC] = x[k * NPC:(k + 1) * NPC]
        fd = np.zeros(EPC_PAD, dtype=np.int64)
        fd[:EPC] = pidx[k * EPC:(k + 1) * EPC]
        fs = np.zeros(EPC_PAD, dtype=np.int64)
        fs[:EPC] = cidx[k * EPC:(k + 1) * EPC]
        # stream position j = edge position within the core's padded slice;
        # output slot (p, i) = (j % 128, j // 128)
        in_maps.append({
            "xs": xsl,
            "qd": _wrap16((fd >> 3).astype(np.int16)),
            "qs": _wrap16((fs >> 3).astype(np.int16)),
            "rd": np.ascontiguousarray(
                (fd & 7).astype(np.float32).reshape(S, 128).T),
            "rs": np.ascontiguousarray(
                (fs & 7).astype(np.float32).reshape(S, 128).T),
            "w": w,
            "scal": scal,
        })

    if _CACHED_NC is None:
        _CACHED_NC = _build_nc()
    res = bass_utils.run_bass_kernel_spmd(
        _CACHED_NC, in_maps, core_ids=list(range(N_CORES))
    )
    global _LAST_RES
    _LAST_RES = res
    outs = []
    for k in range(N_CORES):
        o2d = res.results[k]["out"].reshape(128, S)
        outs.append(o2d.T.reshape(-1)[:EPC])
    return np.concatenate(outs)


# revision 12
# speedup vs baseline: 2.4236x; 2.4236x over previous
"""Edge-parallel GNN message-passing kernel for 8 trn2 NeuronCores.

Computation (see reference):
    p = x @ Wp + bp   [N,1]
    c = x @ Wc + bc   [N,1]
    out[e] = |p[dst[e]] - c[src[e]]| * W1 + b1   for each edge e

Strategy:
  - Node projection is sharded: core k projects nodes [12500k, 12500(k+1)).
    Bias is folded into the final affine ((p+bp)-(c+bc) = p-c+(bp-bc)).
  - An 8-core AllGather shares the per-node projections (800 KB total) so
    every core holds the full (p, c) table g[200704] in DRAM.
  - The table is re-laid out as T8[q, 0:8] = g[8q:8q+8] (rows 256 B apart)
    so the bulk SWDGE dma_gather instruction (256 B elements, int16 row
    indices < 32768) can fetch, for each edge endpoint, the 8-value row
    containing its projection; an 8-wide masked select on DVE picks the
    right value.
  - Edges are sharded contiguously, 75000 per core; per-edge row/offset
    indices are precomputed on the host. The fused sub/abs/affine tail
    runs on DVE/ACT and each core writes its output slice.
"""

import numpy as np

import concourse.bacc as bacc
import concourse.tile as tile
from concourse import bass, mybir
from concourse import bass_utils
from concourse.masks import make_identity

N_CORES = 8
N_NODES = 100000
N_EDGES = 600000
IN_CH = 128

NPC = 12500          # real nodes per core
NPC_PAD = 12544      # padded to 98*128
T_TILES = 98         # node tiles per core
EPC = 75000          # edges per core
S = 587              # free-dim width of edge tiles (128*587 = 75136)
EPC_PAD = 128 * S
G_FLAT = N_CORES * 2 * NPC_PAD   # 200704 table elements
T8_ROWS = G_FLAT // 8            # 25088 rows of 8 values (256B apart)
NI_CHUNK = 8192                  # dma_gather indices per instruction
IDX_COLS = EPC_PAD // 16         # 4696

F32 = mybir.dt.float32
I16 = mybir.dt.int16

_CACHED_NC = None
_LAST_RES = None


def _build_nc():
    nc = bacc.Bacc("TRN2", target_bir_lowering=False, debug=False,
                   num_devices=N_CORES, num_swdge_queues=4)

    xs = nc.dram_tensor("xs", [NPC_PAD, IN_CH], F32, kind="ExternalInput")
    qd = nc.dram_tensor("qd", [128, IDX_COLS], I16, kind="ExternalInput")
    qs = nc.dram_tensor("qs", [128, IDX_COLS], I16, kind="ExternalInput")
    rd = nc.dram_tensor("rd", [128, S], F32, kind="ExternalInput")
    rs = nc.dram_tensor("rs", [128, S], F32, kind="ExternalInput")
    w = nc.dram_tensor("w", [IN_CH, 2], F32, kind="ExternalInput")
    scal = nc.dram_tensor("scal", [128, 16], F32, kind="ExternalInput")
    out = nc.dram_tensor("out", [EPC_PAD], F32, kind="ExternalOutput")
    import os as _os
    _dbg = _os.environ.get("K_DEBUG") == "1"
    if _dbg:
        out_vd = nc.dram_tensor("out_vd", [128, S], F32, kind="ExternalOutput")
        out_pd = nc.dram_tensor("out_pd", [128, S], F32, kind="ExternalOutput")
        out_vs = nc.dram_tensor("out_vs", [128, S], F32, kind="ExternalOutput")
        out_gt = nc.dram_tensor("out_gt", [N_CORES, 2, NPC_PAD], F32,
                                kind="ExternalOutput")
        out_sc = nc.dram_tensor("out_sc", [128, 16], F32, kind="ExternalOutput")

    with tile.TileContext(nc) as tc:
        with (
            tc.tile_pool(name="cst", bufs=1) as cst,
            tc.tile_pool(name="sb", bufs=3) as sb,
            tc.tile_pool(name="edge", bufs=1) as edge,
            tc.tile_pool(name="gat", bufs=3) as gat,
            tc.tile_pool(name="ps", bufs=2, space="PSUM") as ps,
            tc.tile_pool(name="pcps", bufs=1, space="PSUM") as pcps,
            tc.tile_pool(name="dram", bufs=1, space="DRAM") as dram,
        ):
            ident = cst.tile([128, 128], F32)
            make_identity(nc, ident[:])
            w_sb = cst.tile([IN_CH, 2], F32)
            nc.sync.dma_start(out=w_sb[:], in_=w[:])
            scal_sb = cst.tile([128, 16], F32)
            nc.sync.dma_start(out=scal_sb[:], in_=scal[:])

            qd_sb = edge.tile([128, IDX_COLS], I16)
            nc.sync.dma_start(out=qd_sb[:], in_=qd[:])
            qs_sb = edge.tile([128, IDX_COLS], I16)
            nc.sync.dma_start(out=qs_sb[:], in_=qs[:])
            rd_sb = edge.tile([128, S], F32)
            nc.sync.dma_start(out=rd_sb[:], in_=rd[:])
            rs_sb = edge.tile([128, S], F32)
            nc.sync.dma_start(out=rs_sb[:], in_=rs[:])

            # ---- phase 1: project this core's nodes: pc[n, 0:2] = x[n] @ [Wp|Wc]
            J = 7
            G = T_TILES // J
            xs_r = xs.rearrange("(g j p) c -> g p j c", j=J, p=128)
            pc_ps = pcps.tile([128, 2 * T_TILES], F32)
            for g in range(G):
                xt = sb.tile([128, J, IN_CH], F32, tag="xt")
                nc.sync.dma_start(out=xt[:], in_=xs_r[g])
                for j in range(J):
                    t = g * J + j
                    tp = ps.tile([128, 128], F32, tag="tp")
                    nc.tensor.transpose(tp[:], xt[:, j, :], ident[:])
                    x_t = sb.tile([128, 128], F32, tag="x_t")
                    nc.vector.tensor_copy(x_t[:], tp[:])
                    nc.tensor.matmul(
                        out=pc_ps[:, 2 * t:2 * t + 2],
                        lhsT=x_t[:],
                        rhs=w_sb[:],
                        start=True,
                        stop=True,
                    )
            pc_sb = cst.tile([128, 2 * T_TILES], F32)
            nc.vector.tensor_copy(pc_sb[:], pc_ps[:])

            # ---- phase 2: transpose p and c into node-contiguous rows
            bounce = dram.tile([2, NPC_PAD], F32)
            for comp in range(2):
                cp_ps = ps.tile([T_TILES, 128], F32, tag="cp")
                nc.tensor.transpose(
                    cp_ps[:], pc_sb[:, comp::2], ident[:]
                )
                row = sb.tile([T_TILES, 128], F32, tag="row")
                nc.vector.tensor_copy(row[:], cp_ps[:])
                nc.sync.dma_start(
                    out=bounce[comp].rearrange("(t p) -> t p", p=128),
                    in_=row[:],
                )

            # ---- phase 3: all-gather the projection table
            g_tab = dram.tile([N_CORES, 2, NPC_PAD], F32)
            nc.gpsimd.collective_compute(
                "AllGather",
                mybir.AluOpType.bypass,
                replica_groups=[list(range(N_CORES))],
                ins=[bounce.opt()],
                outs=[g_tab.opt()],
            )

            # ---- phase 3b: spread g into 256B-strided rows T8[q,0:8]=g[8q:8q+8]
            # (SBUF-side expansion + one contiguous store: descriptor-cheap)
            t8 = dram.tile([T8_ROWS, 64], F32)
            g_sb = cst.tile([128, G_FLAT // 128], F32)
            nc.sync.dma_start(
                out=g_sb[:],
                in_=g_tab.rearrange("a b (p f) -> p (a b f)", p=1)
                .rearrange("one (p f) -> (one p) f", p=128),
            )
            t8_sb = cst.tile([128, (T8_ROWS // 128) * 64], F32)
            nc.vector.tensor_copy(
                out=t8_sb[:].rearrange("p (r e) -> p r e", e=64)[:, :, 0:8],
                in_=g_sb[:].rearrange("p (r e) -> p r e", e=8),
            )
            nc.sync.dma_start(
                out=t8.rearrange("(p r) e -> p (r e)", p=128),
                in_=t8_sb[:],
            )

            # ---- phase 4+5: bulk-gather endpoint rows, 8-wide select, tail
            val_d = edge.tile([128, S], F32)
            val_s = edge.tile([128, S], F32)
            res = edge.tile([128, S], F32)

            iota_b = scal_sb[:, 0:8]  # cols 0..7 hold 0..7
            n_full = EPC_PAD // NI_CHUNK          # 9 full chunks
            widths = [NI_CHUNK // 128] * n_full   # 64 columns each
            rem = EPC_PAD - n_full * NI_CHUNK
            if rem:
                widths.append(rem // 128)
            i0 = 0
            gather_no = 0
            for wdt in widths:
                ni = wdt * 128
                icol0 = i0 * 8
                for qx_sb, rx_sb, vx in (
                    (qs_sb, rs_sb, val_s),
                    (qd_sb, rd_sb, val_d),
                ):
                    gth = gat.tile([128, NI_CHUNK // 128, 64], F32, tag="gth")
                    nc.gpsimd.dma_gather(
                        out_ap=gth[:, :wdt, :],
                        in_ap=t8[:],
                        idxs_ap=qx_sb[:, icol0:icol0 + wdt * 8],
                        num_idxs=ni,
                        num_idxs_reg=ni,
                        elem_size=64,
                        single_packet=False,
                        queue_num=gather_no % 4,
                    )
                    gather_no += 1
                    msk = gat.tile([128, NI_CHUNK // 128, 8], F32, tag="msk")
                    nc.vector.tensor_tensor(
                        out=msk[:, :wdt, :],
                        in0=iota_b.rearrange("p (one e) -> p one e", one=1).broadcast_to([128, wdt, 8]),
                        in1=rx_sb[:, i0:i0 + wdt].rearrange("p (i one) -> p i one", one=1).broadcast_to([128, wdt, 8]),
                        op=mybir.AluOpType.is_equal,
                    )
                    nc.vector.tensor_tensor(
                        out=msk[:, :wdt, :],
                        in0=msk[:, :wdt, :],
                        in1=gth[:, :wdt, 0:8],
                        op=mybir.AluOpType.mult,
                    )
                    nc.vector.tensor_reduce(
                        out=vx[:, i0:i0 + wdt],
                        in_=msk[:, :wdt, :],
                        axis=mybir.AxisListType.X,
                        op=mybir.AluOpType.add,
                    )
                if _dbg:
                    nc.sync.dma_start(out=out_pd[:, i0:i0 + wdt],
                                      in_=val_d[:, i0:i0 + wdt])
                # tail: |pd - cs + (bp-bc)| * w1 + b1
                sl = slice(i0, i0 + wdt)
                nc.vector.tensor_tensor(
                    out=val_d[:, sl], in0=val_d[:, sl], in1=val_s[:, sl],
                    op=mybir.AluOpType.subtract,
                )
                nc.scalar.activation(
                    out=val_d[:, sl], in_=val_d[:, sl],
                    func=mybir.ActivationFunctionType.Abs,
                    bias=scal_sb[:, 8:9], scale=1.0,
                )
                nc.vector.scalar_tensor_tensor(
                    out=res[:, sl], in0=val_d[:, sl],
                    scalar=scal_sb[:, 9:10],
                    in1=scal_sb[:, 10:11].to_broadcast([128, wdt]),
                    op0=mybir.AluOpType.mult,
                    op1=mybir.AluOpType.add,
                )
                i0 += wdt
            nc.sync.dma_start(
                out=out.rearrange("(p s) -> p s", s=S), in_=res[:]
            )
            if _dbg:
                nc.sync.dma_start(out=out_sc[:], in_=scal_sb[:])
                nc.sync.dma_start(out=out_vd[:], in_=val_d[:])
                nc.sync.dma_start(out=out_vs[:], in_=val_s[:])
                nc.sync.dma_start(out=out_gt[:], in_=g_tab[:])

    nc.compile()
    return nc


def _wrap16(stream):
    """idx j -> [j % 16, j // 16], replicated to all 8 gpsimd core groups."""
    w = stream.reshape(-1, 16).T  # [16, COLS]
    return np.tile(w, (8, 1))


def kernel(x, adjs, Wp, bp, Wc, bc, W1, b1):
    global _CACHED_NC
    x = np.ascontiguousarray(np.asarray(x, dtype=np.float32))
    adjs = np.asarray(adjs)
    Wp = np.asarray(Wp, dtype=np.float32)
    bp = np.asarray(bp, dtype=np.float32)
    Wc = np.asarray(Wc, dtype=np.float32)
    bc = np.asarray(bc, dtype=np.float32)
    W1 = np.asarray(W1, dtype=np.float32)
    b1 = np.asarray(b1, dtype=np.float32)

    src = adjs[0].astype(np.int64)
    dst = adjs[1].astype(np.int64)
    # flat indices into the gathered table g[core, comp, node_in_core]
    pidx = (dst // NPC) * (2 * NPC_PAD) + (dst % NPC)
    cidx = (src // NPC) * (2 * NPC_PAD) + NPC_PAD + (src % NPC)

    w = np.concatenate([Wp, Wc], axis=1)  # [128, 2]
    scal = np.zeros((128, 16), dtype=np.float32)
    scal[:, 0:8] = np.arange(8, dtype=np.float32)[None, :]
    scal[:, 8] = bp[0] - bc[0]
    scal[:, 9] = W1[0, 0]
    scal[:, 10] = b1[0]

    in_maps = []
    for k in range(N_CORES):
        xsl = np.zeros((NPC_PAD, IN_CH), dtype=np.float32)
        xsl[:NPC] = x[k * NPC:(k + 1) * NPC]
        fd = np.zeros(EPC_PAD, dtype=np.int64)
        fd[:EPC] = pidx[k * EPC:(k + 1) * EPC]
        fs = np.zeros(EPC_PAD, dtype=np.int64)
        fs[:EPC] = cidx[k * EPC:(k + 1) * EPC]
        # stream position j = edge position within the core's padded slice;
        # output slot (p, i) = (j % 128, j // 128)
        in_maps.append({
            "xs": xsl,
            "qd": _wrap16((fd >> 3).astype(np.int16)),
            "qs": _wrap16((fs >> 3).astype(np.int16)),
            "rd": np.ascontiguousarray(
                (fd & 7).astype(np.float32).reshape(S, 128).T),
            "rs": np.ascontiguousarray(
                (fs & 7).astype(np.float32).reshape(S, 128).T),
            "w": w,
            "scal": scal,
        })

    if _CACHED_NC is None:
        _CACHED_NC = _build_nc()
    res = bass_utils.run_bass_kernel_spmd(
        _CACHED_NC, in_maps, core_ids=list(range(N_CORES))
    )
    global _LAST_RES
    _LAST_RES = res
    outs = []
    for k in range(N_CORES):
        o2d = res.results[k]["out"].reshape(128, S)
        outs.append(o2d.T.reshape(-1)[:EPC])
    return np.concatenate(outs)
